# revision 17
# baseline (speedup 1.0000x reference)
"""Trainium2 Bass kernel for causal self-attention (B=2, T=2048, C=1024, H=16, D=64).

Sharding: 8 cores = 2 batches x 4 head-groups (4 heads each, 256 channels).
Each core computes, for its (batch b, head-group g):
  - x^T on chip (PE transposes)
  - qT/kT projections in [c, t] layout, v in [t, c] layout
  - causal attention per head with scores computed transposed (ST[s,t]) so
    softmax needs no transposes; denominator folded into the O-matmul via a
    [V | e_h] augmented stationary operand (V68); normalization deferred
    through the (linear) output projection.
  - partial y = att_out @ Wp[g-rows]  (host sums the 4 partials per batch)
Outputs per core: y_part [T, C], k-slice [T, 256], v-slice [T, 256].
Host: sums y partials + bp, concatenates k/v slices (biases applied on device).
"""

import os
import sys

import numpy as np

for _p in ("/root/.axon_site", "/root/.axon_site/_ro/trn_rl_repo", "/opt/trn_rl_repo"):
    if os.path.isdir(_p) and _p not in sys.path:
        sys.path.append(_p)

import concourse.bass as bass
import concourse.bacc as bacc
import concourse.mybir as mybir
import concourse.tile as tile
from concourse import bass_utils
from contextlib import ExitStack

T, C, H, D = 2048, 1024, 16, 64
NCORES = 8
HPC = 4            # heads per core
CPC = HPC * D      # 256 channels per core
NK = C // 128      # 8 cin chunks
NT = T // 128      # 16 t blocks
NTC = T // 512     # 4 t chunks

MMDT = mybir.dt.float32r   # matmul operand dtype (1 cyc/row on PE at N>=256)
FP32 = mybir.dt.float32
AF = mybir.ActivationFunctionType
SM_SCALE = 1.0 / np.sqrt(D)


def build_nc():
    nc = bacc.Bacc("TRN2", target_bir_lowering=False, debug=False)

    x_d = nc.dram_tensor("x", [T, C], MMDT, kind="ExternalInput")
    wq_d = nc.dram_tensor("wq", [C, CPC], MMDT, kind="ExternalInput")
    wk_d = nc.dram_tensor("wk", [C, CPC], MMDT, kind="ExternalInput")
    wv_d = nc.dram_tensor("wv", [C, CPC], MMDT, kind="ExternalInput")
    wp_d = nc.dram_tensor("wp", [64, HPC, C], MMDT, kind="ExternalInput")  # per-head rows
    bq_d = nc.dram_tensor("bq", [CPC], MMDT, kind="ExternalInput")
    bk_d = nc.dram_tensor("bk", [CPC], MMDT, kind="ExternalInput")
    bv_d = nc.dram_tensor("bv", [CPC], MMDT, kind="ExternalInput")
    ident_d = nc.dram_tensor("ident", [128, 128], MMDT, kind="ExternalInput")

    y_d = nc.dram_tensor("y", [T, C], MMDT, kind="ExternalOutput")
    ko_d = nc.dram_tensor("ko", [T, CPC], MMDT, kind="ExternalOutput")
    vo_d = nc.dram_tensor("vo", [T, CPC], MMDT, kind="ExternalOutput")

    x_ap = x_d.ap()
    ko_r = ko_d.ap().rearrange("(q p) c -> p q c", p=128)          # [128, 16, 256]
    vo_r = vo_d.ap().rearrange("(q p) (h e) -> p q h e", p=128, h=HPC)  # [128,16,4,64]

    with tile.TileContext(nc) as tc, ExitStack() as ctx:
        persist = ctx.enter_context(tc.tile_pool(name="persist", bufs=1))

        qT_sb = persist.tile([128, 2, T], MMDT, name="qT_sb")
        kT_sb = persist.tile([128, 2, T], MMDT, name="kT_sb")
        v65_sb = persist.tile([128, NT, HPC, 65], MMDT, name="v65_sb")
        bq_sb = persist.tile([128, 2], MMDT, name="bq_sb")
        bk_sb = persist.tile([128, 2], MMDT, name="bk_sb")
        bv_sb = persist.tile([1, CPC], MMDT, name="bv_sb")
        ones1_sb = persist.tile([1, 128], MMDT, name="ones1_sb")
        ones64_sb = persist.tile([128, 64], MMDT, name="ones64_sb")
        ident_sb = persist.tile([128, 128], MMDT, name="ident_sb")

        nc.sync.dma_start(bq_sb[:, :], bq_d.ap().rearrange("(j p) -> p j", p=128))
        nc.sync.dma_start(bk_sb[:, :], bk_d.ap().rearrange("(j p) -> p j", p=128))
        nc.sync.dma_start(bv_sb[0:1, :], bv_d.ap().rearrange("(o c) -> o c", o=1))
        nc.sync.dma_start(ident_sb[:, :], ident_d.ap())
        # memset on float32r fails the walrus ISA check; write the fp32 bit
        # pattern through a uint32 bitcast instead.
        one_bits = int(np.float32(1.0).view(np.uint32))
        nc.vector.memset(ones1_sb[0:1, :].bitcast(mybir.dt.uint32), one_bits)
        nc.vector.memset(ones64_sb[64:65, :].bitcast(mybir.dt.uint32), one_bits)
        # ones column 64 of V65 (softmax-denominator trick)
        nc.vector.memset(v65_sb[:, :, :, 64:65].bitcast(mybir.dt.uint32), one_bits)

        # ---------------- Phase A+B: transpose x, projections ----------------
        with ExitStack() as s1:
            pA = s1.enter_context(tc.tile_pool(name="pA", bufs=1))
            xload = s1.enter_context(tc.tile_pool(name="xload", bufs=8))
            tp_ps = s1.enter_context(tc.tile_pool(name="tp_ps", bufs=4, space="PSUM"))
            pj_ps = s1.enter_context(tc.tile_pool(name="pj_ps", bufs=2, space="PSUM"))
            kst_p = s1.enter_context(tc.tile_pool(name="kst_p", bufs=2))

            xT_sb = pA.tile([128, NK, T], MMDT, name="xT_sb")
            wq_sb = pA.tile([128, NK, CPC], MMDT, name="wq_sb")
            wk_sb = pA.tile([128, NK, CPC], MMDT, name="wk_sb")
            wv_sb = pA.tile([128, NK, CPC], MMDT, name="wv_sb")

            nc.sync.dma_start(wq_sb[:, :, :], wq_d.ap().rearrange("(j p) c -> p j c", p=128))
            nc.sync.dma_start(wk_sb[:, :, :], wk_d.ap().rearrange("(j p) c -> p j c", p=128))
            nc.sync.dma_start(wv_sb[:, :, :], wv_d.ap().rearrange("(j p) c -> p j c", p=128))

            # transpose x -> xT
            for grp in range(4):
                xts = []
                for ii in range(4):
                    i = grp * 4 + ii
                    xt = xload.tile([128, C], MMDT, name="xt", tag="xt")
                    nc.sync.dma_start(xt[:, :], x_ap[i * 128 : (i + 1) * 128, :])
                    xts.append(xt)
                for kk in range(NK):
                    tp = tp_ps.tile([128, 512], MMDT, name="tp", tag="tp")
                    for ii in range(4):
                        nc.tensor.transpose(
                            tp[:, ii * 128 : (ii + 1) * 128],
                            xts[ii][:, kk * 128 : (kk + 1) * 128],
                            ident_sb[:, :],
                        )
                    nc.vector.tensor_copy(
                        xT_sb[:, kk, grp * 512 : (grp + 1) * 512], tp[:, :]
                    )

            # qT / kT projections: [cout, t] layout
            for w_sb, o_sb, b_sb in ((wq_sb, qT_sb, bq_sb), (wk_sb, kT_sb, bk_sb)):
                for jj in range(2):
                    for n in range(NTC):
                        pp = pj_ps.tile([128, 512], FP32, name="pp", tag="pp")
                        for kk in range(NK):
                            nc.tensor.matmul(
                                pp[:, :],
                                lhsT=w_sb[:, kk, jj * 128 : (jj + 1) * 128],
                                rhs=xT_sb[:, kk, n * 512 : (n + 1) * 512],
                                start=(kk == 0),
                                stop=(kk == NK - 1),
                            )
                        nc.scalar.activation(
                            o_sb[:, jj, n * 512 : (n + 1) * 512],
                            pp[:, :],
                            AF.Identity,
                            bias=b_sb[:, jj : jj + 1],
                        )

            # v projection: [t, cout] layout, straight into V68 (+ bias via K=1 matmul)
            for i in range(NT):
                vp = pj_ps.tile([128, CPC], FP32, name="vp", tag="pp")
                for kk in range(NK):
                    nc.tensor.matmul(
                        vp[:, :],
                        lhsT=xT_sb[:, kk, i * 128 : (i + 1) * 128],
                        rhs=wv_sb[:, kk, :],
                        start=(kk == 0),
                        stop=False,
                    )
                nc.tensor.matmul(
                    vp[:, :],
                    lhsT=ones1_sb[0:1, :],
                    rhs=bv_sb[0:1, :],
                    start=False,
                    stop=True,
                )
                nc.vector.tensor_copy(
                    v65_sb[:, i, :, 0:64],
                    vp[:, :].rearrange("p (h e) -> p h e", h=HPC),
                )

            # k_cache: transpose kT back to [t, c] and DMA out
            for grp in range(4):
                kst = kst_p.tile([128, 4, CPC], MMDT, name="kst", tag="kst")
                for ii in range(4):
                    i = grp * 4 + ii
                    kp = pj_ps.tile([128, CPC], MMDT, name="kp", tag="pp")
                    for jj in range(2):
                        nc.tensor.transpose(
                            kp[:, jj * 128 : (jj + 1) * 128],
                            kT_sb[:, jj, i * 128 : (i + 1) * 128],
                            ident_sb[:, :],
                        )
                    nc.vector.tensor_copy(kst[:, ii, :], kp[:, :])
                nc.sync.dma_start(ko_r[:, grp * 4 : (grp + 1) * 4, :], kst[:, :, :])

            # v_cache out (split per head: DMA APs are limited to 3 dims)
            for h in range(HPC):
                nc.sync.dma_start(vo_r[:, :, h, :], v65_sb[:, :, h, 0:64])

        # ---------------- Phase C: attention ----------------
        with ExitStack() as s2:
            late = s2.enter_context(tc.tile_pool(name="late", bufs=1))
            st_ps = s2.enter_context(tc.tile_pool(name="st_ps", bufs=2, space="PSUM"))
            ot_ps = s2.enter_context(tc.tile_pool(name="ot_ps", bufs=2, space="PSUM"))
            ep_p = s2.enter_context(tc.tile_pool(name="ep_p", bufs=3))
            yst_p = s2.enter_context(tc.tile_pool(name="yst_p", bufs=2))

            ots_sb = late.tile([128, HPC, T], MMDT, name="ots_sb")
            den_sb = late.tile([128, HPC, T], MMDT, name="den_sb")
            wp_sb = late.tile([128, HPC, C], MMDT, name="wp_sb")

            nc.sync.dma_start(wp_sb[0:64, :, :], wp_d.ap())

            for tcx in range(NTC):
                ns = 4 * tcx + 4
                for h in range(HPC):
                    jj, r0 = h // 2, (h % 2) * 64
                    ot = ot_ps.tile([65, 512], FP32, name="ot", tag="ot")
                    sb = 0
                    while sb < ns:
                        gn = min(3, ns - sb)
                        stp = st_ps.tile([128, 3 * 512], FP32, name="stp", tag="st")
                        for u in range(gn):
                            nc.tensor.matmul(
                                stp[:, u * 512 : (u + 1) * 512],
                                lhsT=kT_sb[r0 : r0 + 64, jj, (sb + u) * 128 : (sb + u + 1) * 128],
                                rhs=qT_sb[r0 : r0 + 64, jj, tcx * 512 : (tcx + 1) * 512],
                                start=True,
                                stop=True,
                            )
                        ep = ep_p.tile([128, 3 * 512], MMDT, name="ep", tag="ep")
                        nc.scalar.activation(
                            ep[:, 0 : gn * 512], stp[:, 0 : gn * 512], AF.Exp,
                            scale=float(SM_SCALE),
                        )
                        for u in range(gn):
                            s_blk = sb + u
                            if s_blk >= 4 * tcx:  # diagonal: keep t >= s + delta
                                delta = s_blk * 128 - tcx * 512
                                nc.gpsimd.affine_select(
                                    out=ep[:, u * 512 : (u + 1) * 512],
                                    in_=ep[:, u * 512 : (u + 1) * 512],
                                    compare_op=mybir.AluOpType.is_ge,
                                    fill=0.0,
                                    base=-delta,
                                    pattern=[[1, 512]],
                                    channel_multiplier=-1,
                                )
                            nc.tensor.matmul(
                                ot[:, :],
                                lhsT=v65_sb[:, s_blk, h, :],
                                rhs=ep[:, u * 512 : (u + 1) * 512],
                                start=(s_blk == 0),
                                stop=(s_blk == ns - 1),
                            )
                        sb += gn
                    nc.vector.tensor_copy(
                        ots_sb[0:64, h, tcx * 512 : (tcx + 1) * 512], ot[0:64, :]
                    )
                    nc.vector.tensor_copy(
                        den_sb[64:65, h, tcx * 512 : (tcx + 1) * 512],
                        ot[64:65, :],
                    )

            # ---------------- Phase D: normalize + output projection ----------------
            with nc.allow_low_precision(reason="float32r has float32 layout"):
                nc.vector.reciprocal(den_sb[64:65, :, :], den_sb[64:65, :, :])
            for h in range(HPC):
                for tcx in range(NTC):
                    rp = st_ps.tile([64, 512], FP32, name="rp", tag="st")
                    nc.tensor.matmul(
                        rp[:, :],
                        lhsT=ones64_sb[64:65, :],
                        rhs=den_sb[64:65, h, tcx * 512 : (tcx + 1) * 512],
                        start=True,
                        stop=True,
                    )
                    nc.vector.tensor_mul(
                        ots_sb[0:64, h, tcx * 512 : (tcx + 1) * 512],
                        ots_sb[0:64, h, tcx * 512 : (tcx + 1) * 512],
                        rp[0:64, :],
                    )

            for i in range(NT):
                yst = yst_p.tile([128, C], MMDT, name="yst", tag="yst")
                for co in range(2):
                    yp = ot_ps.tile([128, 512], FP32, name="yp", tag="ot")
                    for h in range(HPC):
                        nc.tensor.matmul(
                            yp[:, :],
                            lhsT=ots_sb[0:64, h, i * 128 : (i + 1) * 128],
                            rhs=wp_sb[0:64, h, co * 512 : (co + 1) * 512],
                            start=(h == 0),
                            stop=(h == HPC - 1),
                        )
                    nc.vector.tensor_copy(yst[:, co * 512 : (co + 1) * 512], yp[:, :])
                nc.sync.dma_start(y_d.ap()[i * 128 : (i + 1) * 128, :], yst[:, :])

    nc.compile()
    return nc


_CACHE = {}


def _get_nc():
    if "nc" not in _CACHE:
        _CACHE["nc"] = build_nc()
    return _CACHE["nc"]


def make_in_maps(x, Wk, bk, Wq, bq, Wv, bv, Wp, bp):
    ident = np.eye(128, dtype=np.float32)
    in_maps = []
    for core in range(NCORES):
        b, g = core // HPC, core % HPC
        cols = slice(g * CPC, (g + 1) * CPC)
        wp_g = np.ascontiguousarray(Wp[cols, :]).reshape(HPC, 64, C).transpose(1, 0, 2)
        in_maps.append(
            {
                "x": np.ascontiguousarray(x[b], np.float32),
                "wq": np.ascontiguousarray(Wq[:, cols], np.float32),
                "wk": np.ascontiguousarray(Wk[:, cols], np.float32),
                "wv": np.ascontiguousarray(Wv[:, cols], np.float32),
                "wp": np.ascontiguousarray(wp_g, np.float32),
                "bq": np.ascontiguousarray(bq[cols], np.float32),
                "bk": np.ascontiguousarray(bk[cols], np.float32),
                "bv": np.ascontiguousarray(bv[cols], np.float32),
                "ident": ident,
            }
        )
    return in_maps


def assemble(results, bp):
    y = np.zeros((2, T, C), np.float32)
    k_cache = np.empty((2, T, C), np.float32)
    v_cache = np.empty((2, T, C), np.float32)
    for core in range(NCORES):
        b, g = core // HPC, core % HPC
        cols = slice(g * CPC, (g + 1) * CPC)
        r = results[core]
        y[b] += r["y"]
        k_cache[b][:, cols] = r["ko"]
        v_cache[b][:, cols] = r["vo"]
    y += np.asarray(bp, np.float32)[None, None, :]
    return y, k_cache, v_cache


def kernel(x, Wk, bk, Wq, bq, Wv, bv, Wp, bp, **kwargs):
    nc = _get_nc()
    in_maps = make_in_maps(x, Wk, bk, Wq, bq, Wv, bv, Wp, bp)
    res = bass_utils.run_bass_kernel_spmd(nc, in_maps, core_ids=list(range(NCORES)))
    return assemble(res.results, bp)


# revision 20
# speedup vs baseline: 114.6273x; 114.6273x over previous
"""Trainium2 Bass kernel for causal self-attention (B=2, T=2048, C=1024, H=16, D=64).

Sharding: 8 cores = 2 batches x 4 head-groups (4 heads each, 256 channels).
Each core computes, for its (batch b, head-group g):
  - x^T on chip (PE transposes)
  - qT/kT projections in [c, t] layout, v in [t, c] layout
  - causal attention per head with scores computed transposed (ST[s,t]) so
    softmax needs no transposes; denominator folded into the O-matmul via a
    [V | e_h] augmented stationary operand (V68); normalization deferred
    through the (linear) output projection.
  - partial y = att_out @ Wp[g-rows]  (host sums the 4 partials per batch)
Outputs per core: y_part [T, C], k-slice [T, 256], v-slice [T, 256].
Host: sums y partials + bp, concatenates k/v slices (biases applied on device).
"""

import os
import sys

import numpy as np

for _p in ("/root/.axon_site", "/root/.axon_site/_ro/trn_rl_repo", "/opt/trn_rl_repo"):
    if os.path.isdir(_p) and _p not in sys.path:
        sys.path.append(_p)

import concourse.bass as bass
import concourse.bacc as bacc
import concourse.mybir as mybir
import concourse.tile as tile
from concourse import bass_utils
from contextlib import ExitStack

T, C, H, D = 2048, 1024, 16, 64
NCORES = 8
HPC = 4            # heads per core
CPC = HPC * D      # 256 channels per core
NK = C // 128      # 8 cin chunks
NT = T // 128      # 16 t blocks
NTC = T // 512     # 4 t chunks

MMDT = mybir.dt.float32r   # matmul operand dtype (1 cyc/row on PE at N>=256)
FP32 = mybir.dt.float32
AF = mybir.ActivationFunctionType
SM_SCALE = 1.0 / np.sqrt(D)


def build_nc(body_reps=1):
    nc = bacc.Bacc("TRN2", target_bir_lowering=False, debug=False)

    x_d = nc.dram_tensor("x", [T, C], MMDT, kind="ExternalInput")
    wq_d = nc.dram_tensor("wq", [C, CPC], MMDT, kind="ExternalInput")
    wk_d = nc.dram_tensor("wk", [C, CPC], MMDT, kind="ExternalInput")
    wv_d = nc.dram_tensor("wv", [C, CPC], MMDT, kind="ExternalInput")
    wp_d = nc.dram_tensor("wp", [64, HPC, C], MMDT, kind="ExternalInput")  # per-head rows
    bq_d = nc.dram_tensor("bq", [CPC], MMDT, kind="ExternalInput")
    bk_d = nc.dram_tensor("bk", [CPC], MMDT, kind="ExternalInput")
    bv_d = nc.dram_tensor("bv", [CPC], MMDT, kind="ExternalInput")
    ident_d = nc.dram_tensor("ident", [128, 128], MMDT, kind="ExternalInput")

    y_d = nc.dram_tensor("y", [T, C], MMDT, kind="ExternalOutput")
    ko_d = nc.dram_tensor("ko", [T, CPC], MMDT, kind="ExternalOutput")
    vo_d = nc.dram_tensor("vo", [T, CPC], MMDT, kind="ExternalOutput")

    x_ap = x_d.ap()
    ko_r = ko_d.ap().rearrange("(q p) c -> p q c", p=128)          # [128, 16, 256]
    vo_r = vo_d.ap().rearrange("(q p) (h e) -> p q h e", p=128, h=HPC)  # [128,16,4,64]

    with tile.TileContext(nc) as tc, ExitStack() as ctx:
        persist = ctx.enter_context(tc.tile_pool(name="persist", bufs=1))

        qT_sb = persist.tile([128, 2, T], MMDT, name="qT_sb")
        kT_sb = persist.tile([128, 2, T], MMDT, name="kT_sb")
        v65_sb = persist.tile([128, NT, HPC, 65], MMDT, name="v65_sb")
        bq_sb = persist.tile([128, 2], MMDT, name="bq_sb")
        bk_sb = persist.tile([128, 2], MMDT, name="bk_sb")
        bv_sb = persist.tile([1, CPC], MMDT, name="bv_sb")
        ones1_sb = persist.tile([1, 128], MMDT, name="ones1_sb")
        ones64_sb = persist.tile([128, 64], MMDT, name="ones64_sb")
        ident_sb = persist.tile([128, 128], MMDT, name="ident_sb")

        nc.sync.dma_start(bq_sb[:, :], bq_d.ap().rearrange("(j p) -> p j", p=128))
        nc.sync.dma_start(bk_sb[:, :], bk_d.ap().rearrange("(j p) -> p j", p=128))
        nc.sync.dma_start(bv_sb[0:1, :], bv_d.ap().rearrange("(o c) -> o c", o=1))
        nc.sync.dma_start(ident_sb[:, :], ident_d.ap())
        # memset on float32r fails the walrus ISA check; write the fp32 bit
        # pattern through a uint32 bitcast instead.
        one_bits = int(np.float32(1.0).view(np.uint32))
        nc.vector.memset(ones1_sb[0:1, :].bitcast(mybir.dt.uint32), one_bits)
        nc.vector.memset(ones64_sb[64:65, :].bitcast(mybir.dt.uint32), one_bits)
        # ones column 64 of V65 (softmax-denominator trick)
        nc.vector.memset(v65_sb[:, :, :, 64:65].bitcast(mybir.dt.uint32), one_bits)

        # ---------------- Phase A+B: transpose x, projections ----------------
        for _rep in range(body_reps):
          with ExitStack() as s1:
            pA = s1.enter_context(tc.tile_pool(name="pA", bufs=1))
            xload = s1.enter_context(tc.tile_pool(name="xload", bufs=8))
            tp_ps = s1.enter_context(tc.tile_pool(name="tp_ps", bufs=4, space="PSUM"))
            pj_ps = s1.enter_context(tc.tile_pool(name="pj_ps", bufs=2, space="PSUM"))
            kst_p = s1.enter_context(tc.tile_pool(name="kst_p", bufs=2))

            xT_sb = pA.tile([128, NK, T], MMDT, name="xT_sb")
            wq_sb = pA.tile([128, NK, CPC], MMDT, name="wq_sb")
            wk_sb = pA.tile([128, NK, CPC], MMDT, name="wk_sb")
            wv_sb = pA.tile([128, NK, CPC], MMDT, name="wv_sb")

            nc.sync.dma_start(wq_sb[:, :, :], wq_d.ap().rearrange("(j p) c -> p j c", p=128))
            nc.sync.dma_start(wk_sb[:, :, :], wk_d.ap().rearrange("(j p) c -> p j c", p=128))
            nc.sync.dma_start(wv_sb[:, :, :], wv_d.ap().rearrange("(j p) c -> p j c", p=128))

            # transpose x -> xT
            for grp in range(4):
                xts = []
                for ii in range(4):
                    i = grp * 4 + ii
                    xt = xload.tile([128, C], MMDT, name="xt", tag="xt")
                    nc.sync.dma_start(xt[:, :], x_ap[i * 128 : (i + 1) * 128, :])
                    xts.append(xt)
                for kk in range(NK):
                    tp = tp_ps.tile([128, 512], MMDT, name="tp", tag="tp")
                    for ii in range(4):
                        nc.tensor.transpose(
                            tp[:, ii * 128 : (ii + 1) * 128],
                            xts[ii][:, kk * 128 : (kk + 1) * 128],
                            ident_sb[:, :],
                        )
                    nc.vector.tensor_copy(
                        xT_sb[:, kk, grp * 512 : (grp + 1) * 512], tp[:, :]
                    )

            # qT / kT projections: [cout, t] layout
            for w_sb, o_sb, b_sb in ((wq_sb, qT_sb, bq_sb), (wk_sb, kT_sb, bk_sb)):
                for jj in range(2):
                    for n in range(NTC):
                        pp = pj_ps.tile([128, 512], FP32, name="pp", tag="pp")
                        for kk in range(NK):
                            nc.tensor.matmul(
                                pp[:, :],
                                lhsT=w_sb[:, kk, jj * 128 : (jj + 1) * 128],
                                rhs=xT_sb[:, kk, n * 512 : (n + 1) * 512],
                                start=(kk == 0),
                                stop=(kk == NK - 1),
                            )
                        nc.scalar.activation(
                            o_sb[:, jj, n * 512 : (n + 1) * 512],
                            pp[:, :],
                            AF.Identity,
                            bias=b_sb[:, jj : jj + 1],
                        )

            # v projection: [t, cout] layout, straight into V68 (+ bias via K=1 matmul)
            for i in range(NT):
                vp = pj_ps.tile([128, CPC], FP32, name="vp", tag="pp")
                for kk in range(NK):
                    nc.tensor.matmul(
                        vp[:, :],
                        lhsT=xT_sb[:, kk, i * 128 : (i + 1) * 128],
                        rhs=wv_sb[:, kk, :],
                        start=(kk == 0),
                        stop=False,
                    )
                nc.tensor.matmul(
                    vp[:, :],
                    lhsT=ones1_sb[0:1, :],
                    rhs=bv_sb[0:1, :],
                    start=False,
                    stop=True,
                )
                nc.vector.tensor_copy(
                    v65_sb[:, i, :, 0:64],
                    vp[:, :].rearrange("p (h e) -> p h e", h=HPC),
                )

            # k_cache: transpose kT back to [t, c] and DMA out
            for grp in range(4):
                kst = kst_p.tile([128, 4, CPC], MMDT, name="kst", tag="kst")
                for ii in range(4):
                    i = grp * 4 + ii
                    kp = pj_ps.tile([128, CPC], MMDT, name="kp", tag="pp")
                    for jj in range(2):
                        nc.tensor.transpose(
                            kp[:, jj * 128 : (jj + 1) * 128],
                            kT_sb[:, jj, i * 128 : (i + 1) * 128],
                            ident_sb[:, :],
                        )
                    nc.vector.tensor_copy(kst[:, ii, :], kp[:, :])
                nc.sync.dma_start(ko_r[:, grp * 4 : (grp + 1) * 4, :], kst[:, :, :])

            # v_cache out (split per head: DMA APs are limited to 3 dims)
            for h in range(HPC):
                nc.sync.dma_start(vo_r[:, :, h, :], v65_sb[:, :, h, 0:64])

          # ---------------- Phase C: attention ----------------
          with ExitStack() as s2:
            late = s2.enter_context(tc.tile_pool(name="late", bufs=1))
            st_ps = s2.enter_context(tc.tile_pool(name="st_ps", bufs=2, space="PSUM"))
            ot_ps = s2.enter_context(tc.tile_pool(name="ot_ps", bufs=2, space="PSUM"))
            ep_p = s2.enter_context(tc.tile_pool(name="ep_p", bufs=3))
            yst_p = s2.enter_context(tc.tile_pool(name="yst_p", bufs=2))

            ots_sb = late.tile([128, HPC, T], MMDT, name="ots_sb")
            den_sb = late.tile([128, HPC, T], MMDT, name="den_sb")
            wp_sb = late.tile([128, HPC, C], MMDT, name="wp_sb")

            nc.sync.dma_start(wp_sb[0:64, :, :], wp_d.ap())

            for tcx in range(NTC):
                ns = 4 * tcx + 4
                for h in range(HPC):
                    jj, r0 = h // 2, (h % 2) * 64
                    ot = ot_ps.tile([65, 512], FP32, name="ot", tag="ot")
                    sb = 0
                    while sb < ns:
                        gn = min(3, ns - sb)
                        stp = st_ps.tile([128, 3 * 512], FP32, name="stp", tag="st")
                        for u in range(gn):
                            nc.tensor.matmul(
                                stp[:, u * 512 : (u + 1) * 512],
                                lhsT=kT_sb[r0 : r0 + 64, jj, (sb + u) * 128 : (sb + u + 1) * 128],
                                rhs=qT_sb[r0 : r0 + 64, jj, tcx * 512 : (tcx + 1) * 512],
                                start=True,
                                stop=True,
                            )
                        ep = ep_p.tile([128, 3 * 512], MMDT, name="ep", tag="ep")
                        nc.scalar.activation(
                            ep[:, 0 : gn * 512], stp[:, 0 : gn * 512], AF.Exp,
                            scale=float(SM_SCALE),
                        )
                        for u in range(gn):
                            s_blk = sb + u
                            if s_blk >= 4 * tcx:  # diagonal: keep t >= s + delta
                                delta = s_blk * 128 - tcx * 512
                                nc.gpsimd.affine_select(
                                    out=ep[:, u * 512 : (u + 1) * 512],
                                    in_=ep[:, u * 512 : (u + 1) * 512],
                                    compare_op=mybir.AluOpType.is_ge,
                                    fill=0.0,
                                    base=-delta,
                                    pattern=[[1, 512]],
                                    channel_multiplier=-1,
                                )
                            nc.tensor.matmul(
                                ot[:, :],
                                lhsT=v65_sb[:, s_blk, h, :],
                                rhs=ep[:, u * 512 : (u + 1) * 512],
                                start=(s_blk == 0),
                                stop=(s_blk == ns - 1),
                            )
                        sb += gn
                    nc.vector.tensor_copy(
                        ots_sb[0:64, h, tcx * 512 : (tcx + 1) * 512], ot[0:64, :]
                    )
                    nc.vector.tensor_copy(
                        den_sb[64:65, h, tcx * 512 : (tcx + 1) * 512],
                        ot[64:65, :],
                    )

            # ---------------- Phase D: normalize + output projection ----------------
            with nc.allow_low_precision(reason="float32r has float32 layout"):
                nc.vector.reciprocal(den_sb[64:65, :, :], den_sb[64:65, :, :])
            for h in range(HPC):
                for tcx in range(NTC):
                    rp = st_ps.tile([64, 512], FP32, name="rp", tag="st")
                    nc.tensor.matmul(
                        rp[:, :],
                        lhsT=ones64_sb[64:65, :],
                        rhs=den_sb[64:65, h, tcx * 512 : (tcx + 1) * 512],
                        start=True,
                        stop=True,
                    )
                    nc.vector.tensor_mul(
                        ots_sb[0:64, h, tcx * 512 : (tcx + 1) * 512],
                        ots_sb[0:64, h, tcx * 512 : (tcx + 1) * 512],
                        rp[0:64, :],
                    )

            for i in range(NT):
                yst = yst_p.tile([128, C], MMDT, name="yst", tag="yst")
                for co in range(2):
                    yp = ot_ps.tile([128, 512], FP32, name="yp", tag="ot")
                    for h in range(HPC):
                        nc.tensor.matmul(
                            yp[:, :],
                            lhsT=ots_sb[0:64, h, i * 128 : (i + 1) * 128],
                            rhs=wp_sb[0:64, h, co * 512 : (co + 1) * 512],
                            start=(h == 0),
                            stop=(h == HPC - 1),
                        )
                    nc.vector.tensor_copy(yst[:, co * 512 : (co + 1) * 512], yp[:, :])
                nc.sync.dma_start(y_d.ap()[i * 128 : (i + 1) * 128, :], yst[:, :])

    nc.compile()
    return nc


_CACHE = {}


def _get_nc():
    if "nc" not in _CACHE:
        _CACHE["nc"] = build_nc()
    return _CACHE["nc"]


def make_in_maps(x, Wk, bk, Wq, bq, Wv, bv, Wp, bp):
    ident = np.eye(128, dtype=np.float32)
    in_maps = []
    for core in range(NCORES):
        b, g = core // HPC, core % HPC
        cols = slice(g * CPC, (g + 1) * CPC)
        wp_g = np.ascontiguousarray(Wp[cols, :]).reshape(HPC, 64, C).transpose(1, 0, 2)
        in_maps.append(
            {
                "x": np.ascontiguousarray(x[b], np.float32),
                "wq": np.ascontiguousarray(Wq[:, cols], np.float32),
                "wk": np.ascontiguousarray(Wk[:, cols], np.float32),
                "wv": np.ascontiguousarray(Wv[:, cols], np.float32),
                "wp": np.ascontiguousarray(wp_g, np.float32),
                "bq": np.ascontiguousarray(bq[cols], np.float32),
                "bk": np.ascontiguousarray(bk[cols], np.float32),
                "bv": np.ascontiguousarray(bv[cols], np.float32),
                "ident": ident,
            }
        )
    return in_maps


def assemble(results, bp):
    y = np.zeros((2, T, C), np.float32)
    k_cache = np.empty((2, T, C), np.float32)
    v_cache = np.empty((2, T, C), np.float32)
    for core in range(NCORES):
        b, g = core // HPC, core % HPC
        cols = slice(g * CPC, (g + 1) * CPC)
        r = results[core]
        y[b] += r["y"]
        k_cache[b][:, cols] = r["ko"]
        v_cache[b][:, cols] = r["vo"]
    y += np.asarray(bp, np.float32)[None, None, :]
    return y, k_cache, v_cache


def kernel(x, Wk, bk, Wq, bq, Wv, bv, Wp, bp, **kwargs):
    nc = _get_nc()
    in_maps = make_in_maps(x, Wk, bk, Wq, bq, Wv, bv, Wp, bp)
    res = bass_utils.run_bass_kernel_spmd(nc, in_maps, core_ids=list(range(NCORES)))
    return assemble(res.results, bp)
